# revision 12
# baseline (speedup 1.0000x reference)
"""Trainium2 Bass kernel for nn_FusedKQnA (sparse attention with learned
queries + depthwise stride-2 conv aggregation).

Math restructuring (vs the reference):
  - k is never materialized: qkT = x^T @ (Wk @ QW) with QW the block-diagonal
    arrangement of the scaled learned queries -> one (128->32) matmul.
  - The global max subtractions inside the two exp() calls cancel exactly
    between numerator and denominator, so they are dropped.
  - The 1024-channel depthwise conv never materializes.  With
    r = 1/sum_den (computed as exp(-ln(den)), same ACT table set) define
        gamma[t,h,ij] = sum_q kern[t,q*8+h] * r[q*8+h,ij] * cost[n_t(ij),q*8+h]
    Then out_pre[(h,c),ij] = sum_t gamma[t,h,ij] * v[n_t(ij),(h,c)]  (256 ch)
    and out = Wout @ out_pre.
  - gamma's q-contraction + broadcast over the 32 channels of each head is a
    small PE matmul per (tap, batch, channel-chunk) with a one-hot*kern
    stationary operand.

Performance structure (v6):
  - x is host-permuted into parity order (rows [0,2..54,1,3..55], cols
    [1,3..55,0,2..54]) so the stride-2 tap views of the cost/v planes become
    stride-1 slices of parity-region planes (DVE packed modes).
  - Batch stacking: the two batches' 32-group cost work lives on SBUF
    partitions 0-31 / 32-63 of ONE plane, so the exp drains, the den conv
    (block-diagonal M=64 matmuls), ln/exp(-x) and the rc multiplies each
    handle both batches in a single op.
  - PE warm-up: dummy matmuls during the initial x DMA hold the PE HAM
    activity monitor busy so real matmuls run at 2.4 GHz, not 1.2.
  - All small weights pre-transposed/pre-cast to bf16 on the host: every
    constant load is a plain DMA.
  - Engine balance: products + first-level adds on DVE, second-level adds +
    finals on GpSimd, drains split Scalar/DVE.

Sharding: pure data parallel over batch: 16 batches -> 8 cores x 2.
"""

import os
from contextlib import ExitStack

import numpy as np

import concourse.bass as bass
import concourse.mybir as mybir
import concourse.tile as tile
from concourse import bacc
from concourse.bass_utils import run_bass_kernel_spmd

# Problem constants (hardcoded per spec nn_FusedKQnA_1726576854813)
N_Q, N_HEADS, KSIZE, STRIDE, PADDING = 4, 4, 3, 2, 1
B, C, H, W = 16, 128, 56, 56
HC = C // N_HEADS            # 32 head channels
HP = N_HEADS * STRIDE        # 8 effective heads
CS = C * STRIDE              # 256
G = N_Q * HP                 # 32 kernel groups
G2 = 2 * G                   # both batches stacked
HO, WO = H // STRIDE, W // STRIDE   # 28, 28
NCORES = 8
BPC = B // NCORES            # batches per core

TAPS = [(di, dj) for di in (-1, 0, 1) for dj in (-1, 0, 1)]
N_STRIPS = 2
RPS = HO // N_STRIPS         # 14 rows per strip

# plane geometry: rows [even 0:28 | odd-slots 28:57] (odd slot 28 = row -1)
PROWS = 57
ROW0 = {0: 0, -1: 28, 1: 29}
# plane cols [Z 0 | O 1:29 | E 29:57]; O slot 1+m = col 2m+1
PCOLS = 57
COL0 = {-1: 0, 1: 1, 0: 29}

N_RT = 8                     # row tiles per batch (7 natural rows each)
RT = H // N_RT               # 7

F32 = mybir.dt.float32
BF16 = mybir.dt.bfloat16

_BUILD_CACHE = {}


def _host_weights(Wk, Wv, Wout, q_param, attn_scale, rpb_table):
    """Precompute all small weight tensors on the host (bf16, pre-laid-out)."""
    import ml_dtypes
    q = q_param.reshape(N_Q, HP, HC).astype(np.float64) * (HC ** -0.5)
    QW = np.zeros((CS, G), np.float64)
    for qi in range(N_Q):
        for h in range(HP):
            QW[h * HC:(h + 1) * HC, qi * HP + h] = q[qi, h]
    wkq = (Wk.astype(np.float64) @ QW).astype(np.float32)        # (128, 32)

    rpb_exp = np.exp(rpb_table.astype(np.float64))               # (9, 32)
    kern_num = (rpb_exp * attn_scale.astype(np.float64))         # (9, 32)

    # block-diagonal den conv weights for both stacked batches: [64, 9, 64]
    denk = np.zeros((G2, KSIZE * KSIZE, G2), np.float32)
    for t in range(KSIZE * KSIZE):
        for b in range(BPC):
            for g in range(G):
                denk[b * G + g, t, b * G + g] = rpb_exp[t, g]

    # gamma-broadcast stationaries [64, (tau*3+grp)*2+chn, 128]; identical
    # for both partition halves so lhsT can sit at base partition b*32
    kmat = np.zeros((G2, 18, 128), np.float32)
    for t in range(KSIZE * KSIZE):
        grp, tau = divmod(t, 3)
        for ch in range(2):
            for g in range(G):
                h = g % HP
                if h // 4 == ch:
                    m0 = (h % 4) * HC
                    idx = (tau * 3 + grp) * 2 + ch
                    kmat[g, idx, m0:m0 + HC] = kern_num[t, g]
                    kmat[G + g, idx, m0:m0 + HC] = kern_num[t, g]

    # woutT pre-arranged to [128, kc, 256]
    woutT = np.ascontiguousarray(
        Wout.T.astype(np.float32).reshape(2, 128, CS).transpose(1, 0, 2))

    return dict(wkq=wkq.astype(ml_dtypes.bfloat16),
                denk=denk.astype(ml_dtypes.bfloat16),
                kmat=kmat.astype(ml_dtypes.bfloat16),
                woutT=woutT.astype(ml_dtypes.bfloat16),
                wv=np.ascontiguousarray(Wv.astype(ml_dtypes.bfloat16)))


def _build_program():
    nc = bacc.Bacc("TRN2", target_bir_lowering=False, debug=False,
                   enable_asserts=False, num_devices=NCORES)

    # x arrives host-permuted: rows [0,2..54 | 1,3..55], cols [1,3..55 | 0,2..54]
    # split into per-(batch, half, pair-of-tiles) chunks for early compute
    x_d = {(b, hf, p): nc.dram_tensor(f"x{b}_{hf}{p}", [C, 2 * RT, W], BF16,
                                      kind="ExternalInput").ap()
           for b in range(BPC) for hf in range(2) for p in range(2)}
    wkq_d = nc.dram_tensor("wkq", [C, G], BF16, kind="ExternalInput").ap()
    wv_d = nc.dram_tensor("wv", [C, CS], BF16, kind="ExternalInput").ap()
    denk_d = nc.dram_tensor("denk", [G2, 9, G2], BF16,
                            kind="ExternalInput").ap()
    kmat_d = nc.dram_tensor("kmat", [G2, 18, 128], BF16,
                            kind="ExternalInput").ap()
    woutT_d = nc.dram_tensor("woutT", [128, 2, CS], BF16,
                             kind="ExternalInput").ap()
    out_d = nc.dram_tensor("out", [BPC, CS, HO, WO], F32,
                           kind="ExternalOutput").ap()

    with tile.TileContext(nc) as tc, ExitStack() as ctx:
        _kernel_body(ctx, tc, out_d, x_d, wkq_d, wv_d, denk_d, kmat_d,
                     woutT_d)

    _pin_act_tables()
    nc.compile()
    return nc


def _pin_act_tables():
    """Force one ACT table set (natural_log_exp_and_others) for Exp+Ln so the
    scheduler doesn't thrash table loads between them."""
    from concourse import hw_specs
    import concourse.bacc as bacc_mod
    if getattr(bacc_mod, "_act_tables_pinned", False):
        return
    orig = hw_specs.get_activation_tables

    def patched(arch):
        tabs = dict(orig(arch))
        keep = "natural_log_exp_and_others"
        for name in list(tabs):
            if name == keep:
                continue
            fns = tabs[name]
            if any(str(f).endswith((".Exp", ".Ln")) for f in fns):
                tabs[name] = type(fns)()
        return tabs

    bacc_mod.get_activation_tables = patched
    bacc_mod._act_tables_pinned = True


def _kernel_body(ctx, tc, out_d, x_d, wkq_d, wv_d, denk_d, kmat_d,
                 woutT_d):
    nc = tc.nc

    consts = ctx.enter_context(tc.tile_pool(name="consts", bufs=1))
    planes = ctx.enter_context(tc.tile_pool(name="planes", bufs=1))
    xpool = ctx.enter_context(tc.tile_pool(name="xpool", bufs=1))
    small = ctx.enter_context(tc.tile_pool(name="small", bufs=2))
    rcpool = ctx.enter_context(tc.tile_pool(name="rcpool", bufs=1))
    prod_pool = ctx.enter_context(tc.tile_pool(name="prod", bufs=4))
    opre_pool = ctx.enter_context(tc.tile_pool(name="opre", bufs=2))
    outs_pool = ctx.enter_context(tc.tile_pool(name="outs", bufs=4))

    ps = ctx.enter_context(tc.tile_pool(name="ps", bufs=2, space="PSUM"))

    # ---- PE warm-up fodder: zero tile, no external deps ----
    wz = consts.tile([128, 512], BF16)
    nc.gpsimd.memset(wz, 0.0)
    for i in range(14):
        warm_ps = ps.tile([128, 512], F32, tag="mm", bufs=2, name="warm_ps")
        nc.tensor.matmul(warm_ps, wz[:, 0:128], wz, start=True, stop=True)

    # tiny dummy exp: pulls the ACT table load off the critical path
    dummy = consts.tile([1, 8], F32)
    nc.scalar.activation(out=dummy, in_=wz[0:1, 0:8],
                         func=mybir.ActivationFunctionType.Exp)

    # ---- constants into SBUF (plain DMAs, host-prepared layouts) ----
    wkq_sb = consts.tile([C, G], BF16)
    nc.sync.dma_start(out=wkq_sb, in_=wkq_d)
    wv_sb = consts.tile([C, CS], BF16)
    nc.sync.dma_start(out=wv_sb, in_=wv_d)

    # ---- x into SBUF in consumption order ----
    x_sb = {}
    for hf in range(2):
        for p in range(2):
            for b in range(BPC):
                k = (b, hf, p)
                x_sb[k] = xpool.tile([C, 2 * RT, W], BF16,
                                     name=f"x{b}_{hf}{p}")
                nc.sync.dma_start(out=x_sb[k], in_=x_d[k])

    denk_sb = consts.tile([G2, 9, G2], BF16)
    nc.sync.dma_start(out=denk_sb, in_=denk_d)
    kmat_sb = consts.tile([G2, 18, 128], BF16)
    nc.sync.dma_start(out=kmat_sb, in_=kmat_d)
    woutT_sb = consts.tile([128, 2, CS], BF16)
    nc.sync.dma_start(out=woutT_sb, in_=woutT_d)

    # ---- persistent parity planes (zero pads set once) ----
    cost_pl = planes.tile([G2, PROWS, PCOLS], BF16, tag="cost", name="cost_pl")
    v_pl = [[planes.tile([128, PROWS, PCOLS], BF16, tag=f"v{b}_{chn}",
                         name=f"v_pl{b}_{chn}") for chn in range(2)]
            for b in range(BPC)]
    for pl in [cost_pl] + [v_pl[b][c] for b in range(BPC) for c in range(2)]:
        nc.gpsimd.memset(pl[:, 28, :], 0.0)    # row -1
        nc.gpsimd.memset(pl[:, :, 0], 0.0)     # col -1 (O slot 0)

    def cview(di, dj):
        r0, c0 = ROW0[di], COL0[dj]
        return cost_pl[:, r0:r0 + 28, c0:c0 + 28]

    def vview(b, chn, di, dj):
        r0, c0 = ROW0[di], COL0[dj]
        return v_pl[b][chn][:, r0:r0 + 28, c0:c0 + 28]

    def dst_rows(rt):
        # row tile rt covers permuted rows 7rt..7rt+6; even tiles (rt<4) map
        # to plane rows 7rt.., odd tiles to plane rows 29+7(rt-4)..
        return 7 * rt if rt < 4 else 29 + 7 * (rt - 4)

    # ---- phase A: per row tile, qk (both batches, stacked) + 4 full-array
    #      v matmuls (the 128x128 v matmuls keep the PE HAM warm); den/r/rc
    #      for output strip s start as soon as the rows they read exist ----
    lden = small.tile([G2, 2, RPS, WO], F32, tag="lden", name="lden")
    r_sb = small.tile([G2, 2, RPS, WO], BF16, tag="rr", name="rr")
    rc9 = {}
    for t in range(9):
        grp, tau = divmod(t, 3)
        rc9[(grp, tau)] = rcpool.tile([G2, 2, RPS, WO], BF16,
                                      tag=f"rc{t}", name=f"rc{t}")

    def emit_den_strip(s):
        den_ps = ps.tile([G2, 512], F32, tag="gam", bufs=3, name="den_ps")
        dv = den_ps[:, :RPS * WO].rearrange("p (a c) -> p a c", a=RPS)
        for t, (di, dj) in enumerate(TAPS):
            nc.tensor.matmul(dv, denk_sb[:, t, :],
                             cview(di, dj)[:, s * RPS:(s + 1) * RPS, :],
                             start=(t == 0), stop=(t == 8))
        dfull = den_ps[:, :RPS * WO].rearrange("p (a c) -> p a c", a=RPS)
        nc.scalar.activation(out=lden[:, s], in_=dfull,
                             func=mybir.ActivationFunctionType.Ln)
        nc.scalar.activation(out=r_sb[:, s], in_=lden[:, s], scale=-1.0,
                             func=mybir.ActivationFunctionType.Exp)
        for t, (di, dj) in enumerate(TAPS):
            grp, tau = divmod(t, 3)
            cvs = cview(di, dj)[:, s * RPS:(s + 1) * RPS, :]
            nc.vector.tensor_mul(rc9[(grp, tau)][:, s], cvs, r_sb[:, s])

    for rt in [0, 1, 4, 5, 6, 2, 3, 7]:
        half, idx = (0, rt) if rt < 4 else (1, rt - 4)
        xk = (half, idx // 2)
        xrows = (idx % 2) * RT
        rd = dst_rows(rt)
        qk_ps = ps.tile([G2, RT, W], F32, tag="gam", bufs=3, name="qk_ps")
        for b in range(BPC):
            nc.tensor.matmul(qk_ps[b * G:(b + 1) * G], wkq_sb,
                             x_sb[(b,) + xk][:, xrows:xrows + RT, :],
                             start=True, stop=True)
        # dst cols [1:57] = [O 1:29 | E 29:57]; src [0:28]=odd, [28:56]=even
        dst = cost_pl[:, rd:rd + RT, 1:57].rearrange(
            "p r (g c) -> p r g c", g=2)
        nc.scalar.activation(out=dst,
                             in_=qk_ps.rearrange("p r (g c) -> p r g c", g=2),
                             func=mybir.ActivationFunctionType.Exp)
        for b in range(BPC):
            for chn in range(2):
                v_ps = ps.tile([128, RT, W], F32, tag="mm", bufs=2,
                               name="v_ps")
                nc.tensor.matmul(v_ps, wv_sb[:, chn * 128:(chn + 1) * 128],
                                 x_sb[(b,) + xk][:, xrows:xrows + RT, :],
                                 start=True, stop=True)
                dstv = v_pl[b][chn][:, rd:rd + RT, 1:57].rearrange(
                    "p r (g c) -> p r g c", g=2)
                srcv = v_ps.rearrange("p r (g c) -> p r g c", g=2)
                n_sc = 2 if rt % 2 == 0 else 1
                if 2 * b + chn < n_sc:
                    nc.scalar.copy(out=dstv, in_=srcv)
                else:
                    nc.vector.tensor_copy(dstv, srcv)
        if rt == 6:
            emit_den_strip(0)      # needs plane rows <=43: tiles 0,1,4,5,6
        elif rt == 7:
            emit_den_strip(1)

    # ---- phases D+E per batch ----
    opre_sb = {}
    for b in range(BPC):
        gsums = {}
        for chn in range(2):
            for grp in range(3):
                gams = [ps.tile([128, 2, 512], F32, tag="gam", bufs=3,
                                name=f"gam_ps{tau}") for tau in range(3)]
                for s in range(N_STRIPS):
                    for tau in range(3):
                        gv = gams[tau][:, s, :RPS * WO].rearrange(
                            "p (a c) -> p a c", a=RPS)
                        nc.tensor.matmul(
                            gv,
                            kmat_sb[b * G:(b + 1) * G,
                                    (tau * 3 + grp) * 2 + chn, :],
                            rc9[(grp, tau)][b * G:(b + 1) * G, s],
                            start=True, stop=True)
                ps_taps = []
                for tau in range(3):
                    t = grp * 3 + tau
                    di, dj = TAPS[t]
                    p_sb = prod_pool.tile([128, 2, RPS, WO], BF16,
                                          tag="p", bufs=12, name=f"p{tau}")
                    gfull = gams[tau][:, :, :RPS * WO].rearrange(
                        "p s (a c) -> p s a c", a=RPS)
                    vv = vview(b, chn, di, dj).rearrange(
                        "p (s a) c -> p s a c", s=2)
                    nc.vector.tensor_mul(p_sb, gfull, vv)
                    ps_taps.append(p_sb)
                gs = prod_pool.tile([128, 2 * RPS * WO], BF16,
                                    tag=f"gs{grp}_{b}_{chn}", bufs=1,
                                    name=f"gs{grp}{b}{chn}")
                flat = [p.rearrange("p s a c -> p (s a c)") for p in ps_taps]
                last_unit = b == 1 and chn == 1
                eng1 = nc.gpsimd if (grp == 1 and not last_unit) else nc.vector
                eng1.tensor_add(gs, flat[0], flat[1])
                eng2 = nc.gpsimd if not last_unit else nc.vector
                eng2.tensor_add(gs, gs, flat[2])
                gsums[(grp, chn)] = gs
            # finals for this chn immediately (frees the tail)
            o_sb = opre_pool.tile([128, 2, RPS, WO], BF16,
                                  tag=f"opre{chn}", name=f"opre{chn}")
            of = o_sb.rearrange("p s a c -> p (s a c)")
            eng = nc.gpsimd if b == 0 and chn == 0 else nc.vector
            eng.tensor_add(of, gsums[(0, chn)], gsums[(1, chn)])
            eng.tensor_add(of, of, gsums[(2, chn)])
            for s in range(N_STRIPS):
                opre_sb[(b, chn, s)] = o_sb[:, s]

        for mo in range(2):
            for s in range(N_STRIPS):
                out_ps = ps.tile([128, RPS, WO], F32, tag="mm",
                                 bufs=2, name="out_ps")
                for kc in range(2):
                    nc.tensor.matmul(out_ps,
                                     woutT_sb[:, kc, mo * 128:(mo + 1) * 128],
                                     opre_sb[(b, kc, s)],
                                     start=(kc == 0), stop=(kc == 1))
                o_final = outs_pool.tile([128, RPS, WO], F32)
                nc.scalar.copy(out=o_final, in_=out_ps)
                nc.sync.dma_start(
                    out=out_d[b, mo * 128:(mo + 1) * 128,
                              s * RPS:(s + 1) * RPS, :],
                    in_=o_final)


def _install_ntff_shim():
    """bass_utils expects antenv.axon_hooks (absent in this checkout); shim it
    with the ctypes NTFF hook from trn_agent_boot so trace=True works."""
    import sys
    import types
    try:
        from antenv.axon_hooks import get_axon_ntff_profile_hook  # noqa: F401
        return
    except ImportError:
        pass
    try:
        from trn_agent_boot.trn_boot import _ntff_profile_via_ctypes
        hook = _ntff_profile_via_ctypes("/opt/axon/libaxon_pjrt.so")
    except Exception:
        hook = None
    mod = types.ModuleType("antenv.axon_hooks")
    mod._hook = hook
    mod.get_axon_ntff_profile_hook = lambda: mod._hook
    mod.set_axon_ntff_profile_hook = lambda h: setattr(mod, "_hook", h)
    sys.modules["antenv.axon_hooks"] = mod


def _get_program():
    if "nc" not in _BUILD_CACHE:
        _BUILD_CACHE["nc"] = _build_program()
    return _BUILD_CACHE["nc"]


# host-side pixel permutation: rows even-first, cols odd-first
_ROW_PERM = np.concatenate([np.arange(0, H, 2), np.arange(1, H, 2)])
_COL_PERM = np.concatenate([np.arange(1, W, 2), np.arange(0, W, 2)])


def kernel(x, Wk, Wv, Wout, q_param, attn_scale, rpb_table):
    import ml_dtypes
    x = np.asarray(x, dtype=np.float32)
    xp = x[:, :, _ROW_PERM][:, :, :, _COL_PERM].astype(ml_dtypes.bfloat16)
    wts = _host_weights(np.asarray(Wk), np.asarray(Wv), np.asarray(Wout),
                        np.asarray(q_param), np.asarray(attn_scale),
                        np.asarray(rpb_table))
    nc = _get_program()

    in_maps = []
    for c in range(NCORES):
        m = dict(wts)
        for b in range(BPC):
            xb = xp[c * BPC + b]
            for hf in range(2):
                for p in range(2):
                    r0 = hf * (H // 2) + p * 14
                    m[f"x{b}_{hf}{p}"] = np.ascontiguousarray(
                        xb[:, r0:r0 + 14])
        in_maps.append(m)

    trace = bool(int(os.environ.get("KERNEL_TRACE", "0")))
    if trace:
        _install_ntff_shim()
    res = run_bass_kernel_spmd(nc, in_maps, core_ids=list(range(NCORES)),
                               trace=trace)
    _BUILD_CACHE["last_results"] = res

    out = np.empty((B, CS, HO, WO), np.float32)
    for c in range(NCORES):
        out[c * BPC:(c + 1) * BPC] = res.results[c]["out"]
    return out
